# revision 5
# baseline (speedup 1.0000x reference)
"""Trainium2 Bass kernel for unscaled dot-product attention.

Shapes (hardcoded): query/key/value [2048, 2, 16, 64] fp32.
  scores = einsum('sbnh,tbnh->bnst', q, k)   (UNscaled)
  probs  = softmax(scores, axis=-1)
  out    = einsum('bnst,tbnh->sbnh', probs, v).reshape(2048, 2, 1024)

Sharding: the 32 (b, n) head-slices are split 4-per-core across 8 cores
(core c -> b = c//4, heads 4*(c%4) .. +4). Each core computes attention
for its 4 heads independently; no cross-device communication.

Device-side strategy (per core, heads processed in 2 pairs):
  - The host packs, per head pair, ONE contiguous SBUF-shaped slab
    [128, 6208] = [Q^T pair | K^T pair | V' blocks] so each pair needs a
    single fully-contiguous DMA (one completion semaphore -> the fp32r
    self-loading matmuls never need more than one sync wait, which is
    all the LDWEIGHTS slot supports).
  - Q^T/K^T are [head*64+h, s]; V' is [t, 66] per t-block per head with
    a ones column (so the PV matmul produces the softmax denominator
    for free) and a zero pad column (fp32r weights need an even count).
  - All matmul operands are float32r (fp32 layout, 11-bit mantissa,
    1 PE cycle/row vs 4 for plain fp32). The host pre-rounds inputs to
    the fp32r grid so DMA-ed bits are already "rounded to FP32r".
  - scores are computed TRANSPOSED: scoresT[t_block, s] = K^T.T @ Q^T
    per 128-t block, two heads packed into the 128 contraction rows of
    the PE array (head A partitions 0-63, head B 64-127, concurrent
    matmuls via row tile_position).
  - exp() runs on the Scalar (ACT) engine straight out of PSUM, with no
    max-subtraction: unscaled fp32 scores max out around +-66, so exp
    stays comfortably inside fp32 range (mathematically identical to
    softmax with max-subtraction).
  - PV accumulates CT[66, s] = V'.T @ expT over the 16 t-blocks in PSUM
    (rows 0-63 context^T, row 64 = denominator, row 65 = padding).
  - Normalization: recip of the denominator row, broadcast across
    partitions with a tiny K=1 fp32 matmul, one DVE multiply, and the
    [64, s] result DMAs straight to the per-head transposed output,
    which the host transposes back.
"""

import numpy as np

SQ, B, NHEADS, HN = 2048, 2, 16, 64
N_CORES = 8
HEADS_PER_CORE = 4
VW = 66                     # V' columns per head (64 V + ones + pad)
SLAB_W = 2 * SQ + 16 * 2 * VW   # 6208

_CACHE = {}


def _round_fp32r(x):
    """Round fp32 array to the fp32r grid (11 explicit mantissa bits,
    round-to-nearest-even, low 12 bits zero)."""
    u = np.ascontiguousarray(x, np.float32).view(np.uint32)
    lsb = (u >> 12) & 1
    u = (u + 0x7FF + lsb) & 0xFFFFF000
    return u.astype(np.uint32).view(np.float32)


def _build_program():
    from contextlib import ExitStack

    import concourse.bacc as bacc
    import concourse.mybir as mybir
    import concourse.tile as tile

    f32 = mybir.dt.float32
    f32r = mybir.dt.float32r
    EXP = mybir.ActivationFunctionType.Exp

    nc = bacc.Bacc("TRN2", target_bir_lowering=False, debug=False,
                   num_devices=N_CORES)

    ins = nc.dram_tensor("ins", [2, 128, SLAB_W], f32r,
                         kind="ExternalInput").ap()
    outT = nc.dram_tensor("outT", [256, SQ], f32, kind="ExternalOutput").ap()

    SCH = 512            # s-chunk processed per inner loop
    NCH = SQ // SCH      # 4 chunks
    NT = SQ // 128       # 16 t-blocks

    with tile.TileContext(nc) as tc, ExitStack() as ctx:
        const_pool = ctx.enter_context(tc.tile_pool(name="const", bufs=1))
        slab_pool = ctx.enter_context(tc.tile_pool(name="slab", bufs=2))
        ex_pool = ctx.enter_context(tc.tile_pool(name="ex", bufs=3))
        fin_pool = ctx.enter_context(tc.tile_pool(name="fin", bufs=2))
        ot_pool = ctx.enter_context(tc.tile_pool(name="ot", bufs=2))
        # PSUM budget (8 banks of [128 x 512 fp32]):
        #   scores 2 bufs x 2 banks = 4, CT 2, bcast 2  -> 8
        ps_sc = ctx.enter_context(tc.tile_pool(name="ps_sc", bufs=2, space="PSUM"))
        ps_ct = ctx.enter_context(tc.tile_pool(name="ps_ct", bufs=1, space="PSUM"))
        ps_bc = ctx.enter_context(tc.tile_pool(name="ps_bc", bufs=1, space="PSUM"))

        ones_col = const_pool.tile([1, 128], f32)
        nc.vector.memset(ones_col[:], 1.0)

        for g in range(2):  # head pairs (heads 2g, 2g+1 of this core)
            slab = slab_pool.tile([128, SLAB_W], f32r, tag="slab")
            nc.sync.dma_start(out=slab[:], in_=ins[g])
            QT2 = slab[:, 0:SQ]
            KT2 = slab[:, SQ:2 * SQ]
            v3 = slab[:, 2 * SQ:].rearrange("p (j c) -> p j c", c=2 * VW)

            for c in range(NCH):
                s0 = c * SCH
                CT = ps_ct.tile([128, 1024], f32, tag="ct")
                for j in range(NT):
                    sc = ps_sc.tile([128, 1024], f32, tag="sc")
                    # scoresT[t, s] per head; A in bank 0, B in bank 1
                    nc.tensor.matmul(
                        sc[:, 0:512],
                        lhsT=KT2[0:64, j * 128:(j + 1) * 128],
                        rhs=QT2[0:64, s0:s0 + SCH],
                        start=True, stop=True)
                    nc.tensor.matmul(
                        sc[:, 512:1024],
                        lhsT=KT2[64:128, j * 128:(j + 1) * 128],
                        rhs=QT2[64:128, s0:s0 + SCH],
                        start=True, stop=True)
                    ex = ex_pool.tile([128, 1024], f32r, tag="ex")
                    nc.scalar.activation(ex[:], sc[:], EXP)
                    nc.tensor.matmul(
                        CT[0:VW, 0:512],
                        lhsT=v3[:, j, 0:VW],
                        rhs=ex[:, 0:512],
                        start=(j == 0), stop=(j == NT - 1))
                    nc.tensor.matmul(
                        CT[0:VW, 512:1024],
                        lhsT=v3[:, j, VW:2 * VW],
                        rhs=ex[:, 512:1024],
                        start=(j == 0), stop=(j == NT - 1))

                # Normalize: rows 0-63 = unnormalized ctx^T, row 64 = denom
                CTs = fin_pool.tile([65, 1024], f32, tag="cts")
                nc.vector.tensor_copy(CTs[:], CT[0:65, :])
                rec = fin_pool.tile([1, 1024], f32, tag="rec")
                nc.vector.reciprocal(rec[:], CTs[64:65, :])
                bc = ps_bc.tile([128, 1024], f32, tag="bc")
                nc.tensor.matmul(bc[:, 0:512], lhsT=ones_col,
                                 rhs=rec[:, 0:512], start=True, stop=True)
                nc.tensor.matmul(bc[:, 512:1024], lhsT=ones_col,
                                 rhs=rec[:, 512:1024], start=True, stop=True)
                OTa = ot_pool.tile([64, 512], f32, tag="ota")
                nc.vector.tensor_mul(OTa[:], CTs[0:64, 0:512], bc[0:64, 0:512])
                OTb = ot_pool.tile([64, 512], f32, tag="otb")
                nc.vector.tensor_mul(OTb[:], CTs[0:64, 512:1024],
                                     bc[0:64, 512:1024])
                nc.sync.dma_start(
                    out=outT[(2 * g) * 64:(2 * g + 1) * 64, s0:s0 + SCH],
                    in_=OTa[:])
                nc.sync.dma_start(
                    out=outT[(2 * g + 1) * 64:(2 * g + 2) * 64, s0:s0 + SCH],
                    in_=OTb[:])
    nc.compile()
    return nc


def get_nc():
    if "nc" not in _CACHE:
        _CACHE["nc"] = _build_program()
    return _CACHE["nc"]


def make_in_maps(query, key, value):
    """Host-side sharding + layout prep. Returns list of per-core input maps."""
    query = np.asarray(query, dtype=np.float32)
    key = np.asarray(key, dtype=np.float32)
    value = np.asarray(value, dtype=np.float32)
    in_maps = []
    for c in range(N_CORES):
        b = c // 4
        n0 = HEADS_PER_CORE * (c % 4)
        q = query[:, b, n0:n0 + 4, :]   # [2048, 4, 64]
        k = key[:, b, n0:n0 + 4, :]
        v = value[:, b, n0:n0 + 4, :]
        # [4, 64, 2048] -> per pair rows
        qt = _round_fp32r(q.transpose(1, 2, 0)).reshape(2, 128, SQ)
        kt = _round_fp32r(k.transpose(1, 2, 0)).reshape(2, 128, SQ)
        # V' [2048, 4, 66] -> [16, 128, 2 pairs, 132] -> [2, 128, 16*132]
        vp = np.concatenate(
            [_round_fp32r(v),
             np.ones((SQ, 4, 1), np.float32),
             np.zeros((SQ, 4, 1), np.float32)], axis=2)
        vp = vp.reshape(16, 128, 2, 2 * VW).transpose(2, 1, 0, 3)
        vp = vp.reshape(2, 128, 16 * 2 * VW)
        slab = np.concatenate([qt, kt, vp], axis=2)  # [2, 128, 6208]
        in_maps.append({"ins": np.ascontiguousarray(slab)})
    return in_maps


def assemble_output(results):
    """results: list of per-core {name: array} dicts -> full [2048, 2, 1024]."""
    out = np.empty((SQ, B, NHEADS, HN), np.float32)
    for c in range(N_CORES):
        b = c // 4
        n0 = HEADS_PER_CORE * (c % 4)
        oT = np.asarray(results[c]["outT"])  # [256, 2048]
        out[:, b, n0:n0 + 4, :] = oT.reshape(4, HN, SQ).transpose(2, 0, 1)
    return out.reshape(SQ, B, NHEADS * HN)


def kernel(query, key, value):
    try:
        from concourse.bass_utils import run_bass_kernel_spmd
    except ImportError:
        import sys
        sys.path.insert(0, "/opt/trn_rl_repo")
        from concourse.bass_utils import run_bass_kernel_spmd

    nc = get_nc()
    in_maps = make_in_maps(query, key, value)
    res = run_bass_kernel_spmd(nc, in_maps, list(range(N_CORES)))
    return assemble_output(res.results)


# revision 8
# speedup vs baseline: 17.1503x; 17.1503x over previous
"""Trainium2 Bass kernel for unscaled dot-product attention.

Shapes (hardcoded): query/key/value [2048, 2, 16, 64] fp32.
  scores = einsum('sbnh,tbnh->bnst', q, k)   (UNscaled)
  probs  = softmax(scores, axis=-1)
  out    = einsum('bnst,tbnh->sbnh', probs, v).reshape(2048, 2, 1024)

Sharding: the 32 (b, n) head-slices are split 4-per-core across 8 cores
(core c -> b = c//4, heads 4*(c%4) .. +4). Each core computes attention
for its 4 heads independently; no cross-device communication.

Device-side strategy (per core, heads processed in 2 pairs):
  - The host packs, per head pair, ONE contiguous SBUF-shaped slab
    [128, 6208] = [Q^T pair | K^T pair | V' blocks] so each pair needs a
    single fully-contiguous DMA (one completion semaphore -> the fp32r
    self-loading matmuls never need more than one sync wait, which is
    all the LDWEIGHTS slot supports).
  - Q^T/K^T are [head*64+h, s]; V' is [t, 66] per t-block per head with
    a ones column (so the PV matmul produces the softmax denominator
    for free) and a zero pad column (fp32r weights need an even count).
  - All matmul operands are float32r (fp32 layout, 11-bit mantissa,
    1 PE cycle/row vs 4 for plain fp32). The host pre-rounds inputs to
    the fp32r grid so DMA-ed bits are already "rounded to FP32r".
  - scores are computed TRANSPOSED: scoresT[t_block, s] = K^T.T @ Q^T
    per 128-t block, two heads packed into the 128 contraction rows of
    the PE array (head A partitions 0-63, head B 64-127, concurrent
    matmuls via row tile_position).
  - exp() runs on the Scalar (ACT) engine straight out of PSUM, with no
    max-subtraction: unscaled fp32 scores max out around +-66, so exp
    stays comfortably inside fp32 range (mathematically identical to
    softmax with max-subtraction).
  - PV accumulates CT[66, s] = V'.T @ expT over the 16 t-blocks in PSUM
    (rows 0-63 context^T, row 64 = denominator, row 65 = padding).
  - Normalization: recip of the denominator row, broadcast across
    partitions with a tiny K=1 fp32 matmul, one DVE multiply, and the
    [64, s] result DMAs straight to the per-head transposed output,
    which the host transposes back.
"""

import numpy as np

SQ, B, NHEADS, HN = 2048, 2, 16, 64
N_CORES = 8
HEADS_PER_CORE = 4
VW = 66                     # V' columns per head (64 V + ones + pad)
SLAB_W = 2 * SQ + 16 * 2 * VW   # 6208

_CACHE = {}


def _round_fp32r(x):
    """Round fp32 array to the fp32r grid (11 explicit mantissa bits,
    round-to-nearest-even, low 12 bits zero)."""
    u = np.ascontiguousarray(x, np.float32).view(np.uint32)
    lsb = (u >> 12) & 1
    u = (u + 0x7FF + lsb) & 0xFFFFF000
    return u.astype(np.uint32).view(np.float32)


def _build_program(reps=1):
    from contextlib import ExitStack

    import concourse.bacc as bacc
    import concourse.mybir as mybir
    import concourse.tile as tile

    f32 = mybir.dt.float32
    f32r = mybir.dt.float32r
    EXP = mybir.ActivationFunctionType.Exp

    nc = bacc.Bacc("TRN2", target_bir_lowering=False, debug=False,
                   num_devices=N_CORES)

    ins = nc.dram_tensor("ins", [2, 128, SLAB_W], f32r,
                         kind="ExternalInput").ap()
    outT = nc.dram_tensor("outT", [256, SQ], f32, kind="ExternalOutput").ap()

    SCH = 512            # s-chunk processed per inner loop
    NCH = SQ // SCH      # 4 chunks
    NT = SQ // 128       # 16 t-blocks

    with tile.TileContext(nc) as tc, ExitStack() as ctx:
        const_pool = ctx.enter_context(tc.tile_pool(name="const", bufs=1))
        slab_pool = ctx.enter_context(tc.tile_pool(name="slab", bufs=2))
        ex_pool = ctx.enter_context(tc.tile_pool(name="ex", bufs=3))
        fin_pool = ctx.enter_context(tc.tile_pool(name="fin", bufs=2))
        ot_pool = ctx.enter_context(tc.tile_pool(name="ot", bufs=2))
        # PSUM budget (8 banks of [128 x 512 fp32]):
        #   scores 2 bufs x 2 banks = 4, CT 2, bcast 2  -> 8
        ps_sc = ctx.enter_context(tc.tile_pool(name="ps_sc", bufs=2, space="PSUM"))
        ps_ct = ctx.enter_context(tc.tile_pool(name="ps_ct", bufs=1, space="PSUM"))
        ps_bc = ctx.enter_context(tc.tile_pool(name="ps_bc", bufs=1, space="PSUM"))

        ones_col = const_pool.tile([1, 128], f32)
        nc.vector.memset(ones_col[:], 1.0)

        rep_cm = tc.For_i(0, reps, 1) if reps != 1 else None
        if rep_cm is not None:
            ctx.enter_context(rep_cm)

        for g in range(2):  # head pairs (heads 2g, 2g+1 of this core)
            slab = slab_pool.tile([128, SLAB_W], f32r, tag="slab")
            nc.sync.dma_start(out=slab[:], in_=ins[g])
            QT2 = slab[:, 0:SQ]
            KT2 = slab[:, SQ:2 * SQ]
            v3 = slab[:, 2 * SQ:].rearrange("p (j c) -> p j c", c=2 * VW)

            for c in range(NCH):
                s0 = c * SCH
                CT = ps_ct.tile([128, 1024], f32, tag="ct")
                for j in range(NT):
                    sc = ps_sc.tile([128, 1024], f32, tag="sc")
                    # scoresT[t, s] per head; A in bank 0, B in bank 1
                    nc.tensor.matmul(
                        sc[:, 0:512],
                        lhsT=KT2[0:64, j * 128:(j + 1) * 128],
                        rhs=QT2[0:64, s0:s0 + SCH],
                        start=True, stop=True)
                    nc.tensor.matmul(
                        sc[:, 512:1024],
                        lhsT=KT2[64:128, j * 128:(j + 1) * 128],
                        rhs=QT2[64:128, s0:s0 + SCH],
                        start=True, stop=True)
                    ex = ex_pool.tile([128, 1024], f32r, tag="ex")
                    nc.scalar.activation(ex[:], sc[:], EXP)
                    nc.tensor.matmul(
                        CT[0:VW, 0:512],
                        lhsT=v3[:, j, 0:VW],
                        rhs=ex[:, 0:512],
                        start=(j == 0), stop=(j == NT - 1))
                    nc.tensor.matmul(
                        CT[0:VW, 512:1024],
                        lhsT=v3[:, j, VW:2 * VW],
                        rhs=ex[:, 512:1024],
                        start=(j == 0), stop=(j == NT - 1))

                # Normalize: rows 0-63 = unnormalized ctx^T, row 64 = denom
                CTs = fin_pool.tile([65, 1024], f32, tag="cts")
                nc.vector.tensor_copy(CTs[:], CT[0:65, :])
                rec = fin_pool.tile([1, 1024], f32, tag="rec")
                nc.vector.reciprocal(rec[:], CTs[64:65, :])
                bc = ps_bc.tile([128, 1024], f32, tag="bc")
                nc.tensor.matmul(bc[:, 0:512], lhsT=ones_col,
                                 rhs=rec[:, 0:512], start=True, stop=True)
                nc.tensor.matmul(bc[:, 512:1024], lhsT=ones_col,
                                 rhs=rec[:, 512:1024], start=True, stop=True)
                OTa = ot_pool.tile([64, 512], f32, tag="ota")
                nc.vector.tensor_mul(OTa[:], CTs[0:64, 0:512], bc[0:64, 0:512])
                OTb = ot_pool.tile([64, 512], f32, tag="otb")
                nc.vector.tensor_mul(OTb[:], CTs[0:64, 512:1024],
                                     bc[0:64, 512:1024])
                nc.sync.dma_start(
                    out=outT[(2 * g) * 64:(2 * g + 1) * 64, s0:s0 + SCH],
                    in_=OTa[:])
                nc.sync.dma_start(
                    out=outT[(2 * g + 1) * 64:(2 * g + 2) * 64, s0:s0 + SCH],
                    in_=OTb[:])
    nc.compile()
    return nc


def get_nc(reps=1):
    key = ("nc", reps)
    if key not in _CACHE:
        _CACHE[key] = _build_program(reps)
    return _CACHE[key]


def make_in_maps(query, key, value):
    """Host-side sharding + layout prep. Returns list of per-core input maps."""
    query = np.asarray(query, dtype=np.float32)
    key = np.asarray(key, dtype=np.float32)
    value = np.asarray(value, dtype=np.float32)
    in_maps = []
    for c in range(N_CORES):
        b = c // 4
        n0 = HEADS_PER_CORE * (c % 4)
        q = query[:, b, n0:n0 + 4, :]   # [2048, 4, 64]
        k = key[:, b, n0:n0 + 4, :]
        v = value[:, b, n0:n0 + 4, :]
        # [4, 64, 2048] -> per pair rows
        qt = _round_fp32r(q.transpose(1, 2, 0)).reshape(2, 128, SQ)
        kt = _round_fp32r(k.transpose(1, 2, 0)).reshape(2, 128, SQ)
        # V' [2048, 4, 66] -> [16, 128, 2 pairs, 132] -> [2, 128, 16*132]
        vp = np.concatenate(
            [_round_fp32r(v),
             np.ones((SQ, 4, 1), np.float32),
             np.zeros((SQ, 4, 1), np.float32)], axis=2)
        vp = vp.reshape(16, 128, 2, 2 * VW).transpose(2, 1, 0, 3)
        vp = vp.reshape(2, 128, 16 * 2 * VW)
        slab = np.concatenate([qt, kt, vp], axis=2)  # [2, 128, 6208]
        in_maps.append({"ins": np.ascontiguousarray(slab)})
    return in_maps


def assemble_output(results):
    """results: list of per-core {name: array} dicts -> full [2048, 2, 1024]."""
    out = np.empty((SQ, B, NHEADS, HN), np.float32)
    for c in range(N_CORES):
        b = c // 4
        n0 = HEADS_PER_CORE * (c % 4)
        oT = np.asarray(results[c]["outT"])  # [256, 2048]
        out[:, b, n0:n0 + 4, :] = oT.reshape(4, HN, SQ).transpose(2, 0, 1)
    return out.reshape(SQ, B, NHEADS * HN)


def kernel(query, key, value):
    try:
        from concourse.bass_utils import run_bass_kernel_spmd
    except ImportError:
        import sys
        sys.path.insert(0, "/opt/trn_rl_repo")
        from concourse.bass_utils import run_bass_kernel_spmd

    nc = get_nc()
    in_maps = make_in_maps(query, key, value)
    res = run_bass_kernel_spmd(nc, in_maps, list(range(N_CORES)))
    return assemble_output(res.results)
